# revision 4
# baseline (speedup 1.0000x reference)
"""Depth-aware forward-warp (bilinear splat) + flow add, on 8 trn2 cores.

Strategy: dense shifted-window splat. Corner offsets (dx, dy) are bounded
(|b| <~ 6 for N(0,1) displacements, measured from the actual inputs on host).
For each active integer offset pair, the contribution is
    v_c(src) * tri(bxr(src)-dx) * tri(byr(src)-dy)  added at dst = src+(dx,dy)
with tri(u) = relu(1-|u|)  (exactly the bilinear corner weights + corner
selection + validity masking in one formula).

Layout per chunk (one y-third of one image):
    partitions p = x % 128, free f = xblk*YS + s,  x = xblk*128 + p
    s = y slot (6 halo + 180 rows + 6 halo), YS=192, XB=8 (cols 960..1023 pad)
x-shifts (partition dim) are done by DMA-copying shifted planes (per dx);
y-shifts are free-dim AP offsets with clipped ranges. Weight planes are built
on the scalar (ACT) engine, products/accumulation on DVE.
"""
import sys
sys.path.insert(0, '/opt/trn_rl_repo')
import numpy as np
import concourse.bacc as bacc
import concourse.mybir as mybir
from concourse.tile import TileContext
from concourse.bass_utils import run_bass_kernel_spmd

AF = mybir.ActivationFunctionType
ALU = mybir.AluOpType
DT = mybir.dt.float32

B, H, W = 16, 540, 960
NCORES = 8
IPC = B // NCORES          # images per core (2)
TH = 3                     # y-thirds per image
CH = IPC * TH              # chunks per core (6)
TR = H // TH               # rows per third (180)
HALO = 6                   # max |dy|,|dx| supported by padding
YS = TR + 2 * HALO         # y slots per xblk (192)
XB = 8                     # x blocks (1024 col slots, 960 real)
PP = 128
F = XB * YS                # free els per plane (1536)
PADVAL = np.float32(1.0e9)
EPS = np.float32(1e-6)

_CACHE = {}


def _marshal(plane_b, third, pad):
    """plane_b: [H, W] -> [128, F] chunk plane for given y-third."""
    out = np.full((YS, XB * PP), pad, dtype=np.float32)
    r0 = third * TR - HALO
    lo, hi = max(0, r0), min(H, r0 + YS)
    out[lo - r0:hi - r0, :W] = plane_b[lo:hi, :]
    # [s, xblk, p] -> [p, xblk, s]
    return np.ascontiguousarray(out.reshape(YS, XB, PP).transpose(2, 1, 0).reshape(PP, F))


def _unmarshal(chunk, third, img_plane):
    """chunk [128, F] -> write rows of img_plane [H, W]."""
    t = chunk.reshape(PP, XB, YS).transpose(2, 1, 0).reshape(YS, XB * PP)
    img_plane[third * TR:(third + 1) * TR, :] = t[HALO:HALO + TR, :W]


def _active_sets(bxr, byr):
    """Per chunk-slot active (dx -> sorted dys), unioned across cores.

    bxr/byr: [B, H, W] rounded displacements."""
    sets = [dict() for _ in range(CH)]
    for b in range(B):
        k_img = b % IPC
        for t in range(TH):
            k = k_img * TH + t
            lo, hi = max(0, t * TR - HALO), min(H, t * TR + TR + HALO)
            fx = np.floor(bxr[b, lo:hi]).astype(np.int64)
            fy = np.floor(byr[b, lo:hi]).astype(np.int64)
            # offsets per pixel: {fx, fx+1} x {fy, fy+1}
            pairs = set()
            for ox in (0, 1):
                for oy in (0, 1):
                    h2 = np.unique((fx + ox) * 1000 + (fy + oy))
                    pairs.update(int(v) for v in h2)
            d = sets[k]
            for v in pairs:
                dx, dy = v // 1000, v % 1000
                if dy > 500:
                    dy -= 1000
                    dx += 1
                if abs(dx) > HALO or abs(dy) > HALO:
                    # clamp: contributions outside supported window would be
                    # lost; HALO=6 covers N(0,1) data (max |b| ~5.5). Assert.
                    raise ValueError(f"displacement out of range: dx={dx} dy={dy}")
                d.setdefault(dx, set()).add(dy)
    return tuple(
        tuple(sorted((dx, tuple(sorted(dys))) for dx, dys in s.items()))
        for s in sets
    )


def _register_consts(nc):
    vals = {1.0, 0.0}
    for d in range(-HALO, HALO + 1):
        vals.add(float(-d))
    for v in vals:
        key = (DT, float(v))
        if key in nc.const_aps.aps:
            continue
        t = nc.alloc_sbuf_tensor(f"constf32_{v}", [PP, 1], DT)
        nc.gpsimd.memset(t.ap(), float(v))
        nc.const_aps.aps[key] = t.ap()


def _shift_x(nc, dst, src, dx, zeros):
    """dst[p, xb, s] = src[(p-dx) mod..., xb-carry, s], zero where no source.

    dst/src are [128, XB, YS] tiles; zeros is a [128, 1, YS] zero tile.
    dx != 0. Corner fills use DMA (engine APs need 32-aligned partition
    bases; DMA has no such restriction)."""
    a = abs(dx)
    if dx > 0:
        # slice 1: dst partitions [dx:128] <- src [0:128-dx], same xblk
        nc.sync.dma_start(out=dst[dx:PP, :, :], in_=src[0:PP - dx, :, :])
        # slice 2: dst [0:dx] at xblk>=1 <- src [128-dx:128] at xblk-1
        nc.sync.dma_start(out=dst[0:dx, 1:XB, :], in_=src[PP - dx:PP, 0:XB - 1, :])
        # corner: dst [0:dx] at xblk 0 has no source (x-dx < 0)
        nc.sync.dma_start(out=dst[0:dx, 0:1, :], in_=zeros[0:dx, :, :])
    else:
        nc.sync.dma_start(out=dst[0:PP - a, :, :], in_=src[a:PP, :, :])
        nc.sync.dma_start(out=dst[PP - a:PP, 0:XB - 1, :], in_=src[0:a, 1:XB, :])
        nc.sync.dma_start(out=dst[PP - a:PP, XB - 1:XB, :], in_=zeros[PP - a:PP, :, :])


def build_program(active, reps=1, n_cores=NCORES):
    nc = bacc.Bacc(trn_type="TRN2", debug=False, num_devices=n_cores)
    _register_consts(nc)

    def param(name, out=False):
        return nc.declare_dram_parameter(name, [CH, PP, F], DT, isOutput=out)

    bxr_d, byr_d = param("bxr"), param("byr")
    fbx_d, fby_d, dep_d = param("fbx"), param("fby"), param("dep")
    fax_d, fay_d = param("fax"), param("fay")
    outx_d, outy_d = param("outx", True), param("outy", True)

    with TileContext(nc) as tc:
        from contextlib import ExitStack
        with ExitStack() as ctx:
            z_pool = ctx.enter_context(tc.tile_pool(name="z", bufs=1))
            zeros = z_pool.tile([PP, 1, YS], DT, tag="zeros", name="zeros_t")
            nc.vector.memset(zeros[:], 0.0)
            io_pool = ctx.enter_context(tc.tile_pool(name="io", bufs=1))
            v_pool = ctx.enter_context(tc.tile_pool(name="v", bufs=1))
            acc_pool = ctx.enter_context(tc.tile_pool(name="acc", bufs=1))
            sh_pool = ctx.enter_context(tc.tile_pool(name="sh", bufs=2))
            w_pool = ctx.enter_context(tc.tile_pool(name="w", bufs=1))
            w2_pool = ctx.enter_context(tc.tile_pool(name="w2", bufs=2))
            t_pool = ctx.enter_context(tc.tile_pool(name="t", bufs=1))

            def chunk_body(k):
                SH3 = [PP, XB, YS]
                bxr = io_pool.tile(SH3, DT, tag="bxr", name="bxr_t")
                byr = io_pool.tile(SH3, DT, tag="byr", name="byr_t")
                fbx = io_pool.tile(SH3, DT, tag="fbx", name="fbx_t")
                fby = io_pool.tile(SH3, DT, tag="fby", name="fby_t")
                dep = io_pool.tile(SH3, DT, tag="dep", name="dep_t")
                nc.sync.dma_start(out=bxr[:], in_=bxr_d[k].rearrange("p (xb s) -> p xb s", xb=XB))
                nc.sync.dma_start(out=byr[:], in_=byr_d[k].rearrange("p (xb s) -> p xb s", xb=XB))
                nc.sync.dma_start(out=fbx[:], in_=fbx_d[k].rearrange("p (xb s) -> p xb s", xb=XB))
                nc.sync.dma_start(out=fby[:], in_=fby_d[k].rearrange("p (xb s) -> p xb s", xb=XB))
                nc.sync.dma_start(out=dep[:], in_=dep_d[k].rearrange("p (xb s) -> p xb s", xb=XB))

                # v2 = exp(-dep); v0 = fbx*v2; v1 = fby*v2
                v0 = v_pool.tile(SH3, DT, tag="v0", name="v0_t")
                v1 = v_pool.tile(SH3, DT, tag="v1", name="v1_t")
                v2 = v_pool.tile(SH3, DT, tag="v2", name="v2_t")
                nc.scalar.activation(v2[:], dep[:], AF.Exp, bias=0.0, scale=-1.0)
                nc.vector.tensor_mul(v0[:], fbx[:], v2[:])
                nc.vector.tensor_mul(v1[:], fby[:], v2[:])

                accs = [acc_pool.tile(SH3, DT, tag=f"acc{c}", name=f"acc{c}_t") for c in range(3)]
                for a in accs:
                    nc.vector.memset(a[:], 0.0)

                for dx, dys in active[k]:
                    # Tx = relu(1 - |bxr - dx|)
                    u = w_pool.tile(SH3, DT, tag="ux", name="ux_t")
                    tx = w_pool.tile(SH3, DT, tag="tx", name="tx_t")
                    nc.scalar.activation(u[:], bxr[:], AF.Abs, bias=float(-dx), scale=1.0)
                    nc.scalar.activation(tx[:], u[:], AF.Relu, bias=1.0, scale=-1.0)
                    # P_c = v_c * Tx  (unshifted)
                    ps = [t_pool.tile(SH3, DT, tag=f"p{c}", name=f"p{c}_t") for c in range(3)]
                    for c in range(3):
                        nc.vector.tensor_mul(ps[c][:], (v0, v1, v2)[c][:], tx[:])
                    if dx == 0:
                        pss, byrs = ps, byr
                    else:
                        pss = [sh_pool.tile(SH3, DT, tag=f"ps{c}", name=f"ps{c}_t") for c in range(3)]
                        byrs = sh_pool.tile(SH3, DT, tag="byrs", name="byrs_t")
                        for c in range(3):
                            _shift_x(nc, pss[c], ps[c], dx, zeros)
                        _shift_x(nc, byrs, byr, dx, zeros)
                    for dy in dys:
                        uy = w2_pool.tile(SH3, DT, tag="uy", name="uy_t")
                        ty = w2_pool.tile(SH3, DT, tag="ty", name="ty_t")
                        nc.scalar.activation(uy[:], byrs[:], AF.Abs, bias=float(-dy), scale=1.0)
                        nc.scalar.activation(ty[:], uy[:], AF.Relu, bias=1.0, scale=-1.0)
                        s0a = max(0, dy)
                        s0t = max(0, -dy)
                        cnt = YS - abs(dy)
                        for c in range(3):
                            tm = t_pool.tile(SH3, DT, tag=f"tm{c}", name=f"tm{c}_t")
                            nc.vector.tensor_mul(tm[:], pss[c][:], ty[:])
                            nc.vector.tensor_add(
                                accs[c][:, :, s0a:s0a + cnt],
                                accs[c][:, :, s0a:s0a + cnt],
                                tm[:, :, s0t:s0t + cnt],
                            )

                # tail: out = where(den>eps, acc/max(den,eps), 0) + flowAB
                fax = io_pool.tile(SH3, DT, tag="fbx", name="fax_t")
                fay = io_pool.tile(SH3, DT, tag="fby", name="fay_t")
                nc.sync.dma_start(out=fax[:], in_=fax_d[k].rearrange("p (xb s) -> p xb s", xb=XB))
                nc.sync.dma_start(out=fay[:], in_=fay_d[k].rearrange("p (xb s) -> p xb s", xb=XB))
                mask = w_pool.tile(SH3, DT, tag="ux", name="mask_t")
                mx = w_pool.tile(SH3, DT, tag="tx", name="mx_t")
                rec = w2_pool.tile(SH3, DT, tag="uy", name="rec_t")
                nc.vector.tensor_scalar(mask[:], accs[2][:], float(EPS), None, ALU.is_gt)
                nc.vector.tensor_scalar(mx[:], accs[2][:], float(EPS), None, ALU.max)
                nc.vector.reciprocal(rec[:], mx[:])
                for c, (fa, od) in enumerate(((fax, outx_d), (fay, outy_d))):
                    w1 = t_pool.tile(SH3, DT, tag=f"tm{c}", name=f"w1_{c}_t")
                    w2 = t_pool.tile(SH3, DT, tag=f"p{c}", name=f"w2_{c}_t")
                    w3 = t_pool.tile(SH3, DT, tag=f"tm{c}", name=f"w3_{c}_t")
                    nc.vector.tensor_mul(w1[:], accs[c][:], rec[:])
                    nc.vector.tensor_mul(w2[:], w1[:], mask[:])
                    nc.vector.tensor_add(w3[:], w2[:], fa[:])
                    nc.sync.dma_start(out=od[k].rearrange("p (xb s) -> p xb s", xb=XB), in_=w3[:])

            if reps == 1:
                for k in range(CH):
                    chunk_body(k)
            else:
                with tc.For_i(0, reps, 1):
                    for k in range(CH):
                        chunk_body(k)
    nc.finalize()
    return nc


def _prepare(flowAB, back_flowAB, flowBC, imgB_depth):
    """Host marshaling. Returns (active, in_maps)."""
    flowAB = np.asarray(flowAB, dtype=np.float32)
    back = np.asarray(back_flowAB, dtype=np.float32)
    fbc = np.asarray(flowBC, dtype=np.float32)
    dep = np.asarray(imgB_depth, dtype=np.float32)

    xx = np.arange(W, dtype=np.float32)[None, :]
    yy = np.arange(H, dtype=np.float32)[:, None]
    # rounded displacements reproducing reference's fl(x+bx)-x exactly
    bxr = (xx + back[:, 0]) - xx
    byr = (yy + back[:, 1]) - yy

    active = _active_sets(bxr, byr)

    in_maps = []
    for core in range(NCORES):
        m = {n: np.empty((CH, PP, F), np.float32) for n in
             ("bxr", "byr", "fbx", "fby", "dep", "fax", "fay")}
        for ki in range(IPC):
            b = core * IPC + ki
            for t in range(TH):
                k = ki * TH + t
                m["bxr"][k] = _marshal(bxr[b], t, PADVAL)
                m["byr"][k] = _marshal(byr[b], t, PADVAL)
                m["fbx"][k] = _marshal(fbc[b, 0], t, 0.0)
                m["fby"][k] = _marshal(fbc[b, 1], t, 0.0)
                m["dep"][k] = _marshal(dep[b, 0], t, 0.0)
                m["fax"][k] = _marshal(flowAB[b, 0], t, 0.0)
                m["fay"][k] = _marshal(flowAB[b, 1], t, 0.0)
        in_maps.append(m)
    return active, in_maps


def kernel(flowAB, back_flowAB, flowBC, imgB_depth):
    active, in_maps = _prepare(flowAB, back_flowAB, flowBC, imgB_depth)
    if active not in _CACHE:
        _CACHE[active] = build_program(active)
    nc = _CACHE[active]
    res = run_bass_kernel_spmd(nc, in_maps, core_ids=list(range(NCORES)))
    out = np.empty((B, 2, H, W), np.float32)
    for core in range(NCORES):
        r = res.results[core]
        for ki in range(IPC):
            b = core * IPC + ki
            for t in range(TH):
                k = ki * TH + t
                _unmarshal(r["outx"][k], t, out[b, 0])
                _unmarshal(r["outy"][k], t, out[b, 1])
    return out


if __name__ == "__main__":
    # quick self-test against the reference
    sys.path.insert(0, '/root/problem')
    import importlib.util
    spec = importlib.util.spec_from_file_location("reference", "/root/problem/reference.py")
    ref = importlib.util.module_from_spec(spec)
    spec.loader.exec_module(ref)
    inputs = {k: np.asarray(v) for k, v in ref.setup_inputs().items()}
    expected = np.asarray(ref.reference(**inputs))
    got = kernel(**inputs)
    err = np.abs(got - expected)
    rel = err.max() / (np.abs(expected).max() + 1e-30)
    print(f"abs max err: {err.max():.3e}  rel: {rel:.3e}")
